# revision 3
# baseline (speedup 1.0000x reference)
"""Causal attention layer (K=V=x@W^T, Q=x, residual) on 8 trn2 NeuronCores.

Sharding: per batch (2), query 128-row blocks are dealt round-robin to 4
cores (core j of a batch owns blocks j, j+4, ..., j+28).  Each core runs an
identical SPMD instruction stream over 8 "slots"; slot s is the core's s-th
q-block and statically attends k-tiles 0..s (512 cols each).  The only
per-core data differences are the DMA'd inputs (its q rows + a [128,512]
additive mask for the diagonal k-tile, whose in-tile diagonal offset j*128
is slot-independent).

Per core on-chip: K^T = W @ x^T projected on the fly (f32r matmuls), flash
style k-tile-major loop, softmax with no max-subtraction (scores are
bounded; ACT exp is accurate to beyond +70 and flushes <-88 to 0), PE
transposes for K-natural and P^T, PV accumulated in SBUF, final 1/l scale +
residual.
"""

import os
import sys

import numpy as np

if "/opt/trn_rl_repo" not in sys.path:
    sys.path.insert(0, "/opt/trn_rl_repo")

B, N_CTX, D = 2, 4096, 512
P = 128
N_CORES = 8
N_SLOTS = 8  # q-blocks (128 rows) per core
N_KT = 8  # k tiles (512 cols) per batch
QROWS = N_SLOTS * P  # 1024 q rows per core
MASK_VAL = -1.0e30

_CACHE = {}

# Set to True (e.g. from test.py) to capture an NTFF profile; the measured
# max-core exec time lands in kernel.last_exec_ns.
TRACE = False
last_exec_ns = None


def _install_ntff_shim():
    """antenv.axon_hooks is absent in this image; register a stand-in so
    run_bass_kernel_spmd(trace=True) can reach the axon NTFF profiler."""
    import types

    if "antenv.axon_hooks" in sys.modules:
        return
    m = types.ModuleType("antenv.axon_hooks")
    state = {"hook": None}
    m.set_axon_ntff_profile_hook = lambda h: state.__setitem__("hook", h)
    m.get_axon_ntff_profile_hook = lambda: state["hook"]
    sys.modules["antenv.axon_hooks"] = m
    try:
        from trn_agent_boot.trn_boot import _ntff_profile_via_ctypes

        m.set_axon_ntff_profile_hook(
            _ntff_profile_via_ctypes("/opt/axon/libaxon_pjrt.so")
        )
    except Exception:
        pass


def _build():
    import concourse.mybir as mybir
    from concourse import bacc
    from concourse.masks import make_identity
    from concourse.tile import TileContext

    f32 = mybir.dt.float32
    f32r = mybir.dt.float32r
    Exp = mybir.ActivationFunctionType.Exp
    AX = mybir.AxisListType.X

    nc = bacc.Bacc("TRN2", target_bir_lowering=False)
    xqT = nc.dram_tensor("xqT", [D, QROWS], f32r, kind="ExternalInput")
    xq = nc.dram_tensor("xq", [QROWS, D], f32, kind="ExternalInput")
    xkT = nc.dram_tensor("xkT", [D, N_CTX], f32r, kind="ExternalInput")
    WT = nc.dram_tensor("WT", [D, D], f32r, kind="ExternalInput")
    mask = nc.dram_tensor("mask", [P, 512], f32, kind="ExternalInput")
    out = nc.dram_tensor("out", [QROWS, D], f32, kind="ExternalOutput")

    xqT_r = xqT.rearrange("(o p) q -> p o q", p=P)  # [128, 4, 1024]
    xkT_r = xkT.rearrange("(o p) n -> p o n", p=P)  # [128, 4, 4096]
    WT_r = WT.rearrange("(o p) e -> p o e", p=P)  # [128, 4, 512]
    xq_r = xq.rearrange("(s p) e -> p s e", p=P)  # [128, 8, 512]
    out_r = out.rearrange("(s p) e -> p s e", p=P)

    with TileContext(nc) as tc:
        with (
            tc.tile_pool(name="const", bufs=1) as constp,
            tc.tile_pool(name="kts", bufs=1) as ktp,
            tc.tile_pool(name="xk", bufs=3) as xkp,
            tc.tile_pool(name="work", bufs=3) as workp,
            tc.tile_pool(name="acc", bufs=1) as accp,
            tc.tile_pool(name="mm_ps", bufs=2, space="PSUM") as mmps,
            tc.tile_pool(name="tr_ps", bufs=2, space="PSUM") as trps,
            tc.tile_pool(name="sc_ps", bufs=2, space="PSUM") as scps,
            tc.tile_pool(name="pv_ps", bufs=2, space="PSUM") as pvps,
        ):
            wt_s = constp.tile([P, 4, D], f32r)
            nc.sync.dma_start(wt_s[:], WT_r)
            xqT_s = constp.tile([P, 4, QROWS], f32r)
            nc.sync.dma_start(xqT_s[:], xqT_r)
            xq_s = constp.tile([P, N_SLOTS, D], f32)
            nc.sync.dma_start(xq_s[:], xq_r)
            mask_s = constp.tile([P, 512], f32)
            nc.sync.dma_start(mask_s[:], mask[:])
            identf = constp.tile([P, P], f32)
            make_identity(nc, identf[:])
            ident = constp.tile([P, P], f32r)
            nc.vector.tensor_copy(ident[:], identf[:])

            KT = ktp.tile([P, 4, N_CTX], f32r)  # K^T resident, 64 KB/partition
            outacc = accp.tile([P, N_SLOTS, D], f32)
            lacc = accp.tile([P, N_SLOTS], f32)

            for kt in range(N_KT):
                xk_t = xkp.tile([P, 4, 512], f32r, tag="xk")
                nc.sync.dma_start(xk_t[:], xkT_r[:, :, kt * 512 : (kt + 1) * 512])
                # K^T tile: psum[f 128, n 512] = W[f,:] @ x^T[:, ntile]
                for fc in range(4):
                    ps = mmps.tile([P, 512], f32, tag="mm")
                    for dc in range(4):
                        nc.tensor.matmul(
                            ps[:],
                            wt_s[:, dc, fc * P : (fc + 1) * P],
                            xk_t[:, dc, :],
                            start=(dc == 0),
                            stop=(dc == 3),
                        )
                    nc.vector.tensor_copy(KT[:, fc, kt * 512 : (kt + 1) * 512], ps[:])
                # K natural [k 128, f 512] per 128-block, via PE transpose
                kn_t = workp.tile([P, 4, 512], f32r, tag="knat")
                for kb in range(4):
                    pst = trps.tile([P, 512], f32r, tag="tr")
                    for fc in range(4):
                        nc.tensor.transpose(
                            pst[:, fc * P : (fc + 1) * P],
                            KT[:, fc, kt * 512 + kb * P : kt * 512 + (kb + 1) * P],
                            ident[:],
                        )
                    nc.vector.tensor_copy(kn_t[:, kb, :], pst[:])
                for s in range(kt, N_SLOTS):
                    # scores psum [q 128, k 512]
                    ps_s = scps.tile([P, 512], f32, tag="sc")
                    for fc in range(4):
                        nc.tensor.matmul(
                            ps_s[:],
                            xqT_s[:, fc, s * P : (s + 1) * P],
                            KT[:, fc, kt * 512 : (kt + 1) * 512],
                            start=(fc == 0),
                            stop=(fc == 3),
                        )
                    # P = exp(S (+ mask)), rowsum into lt
                    p_t = workp.tile([P, 512], f32r, tag="p")
                    lt = workp.tile([P, 1], f32, tag="lt")
                    if s == kt:
                        s_t = workp.tile([P, 512], f32, tag="sm")
                        nc.vector.tensor_add(s_t[:], ps_s[:], mask_s[:])
                        nc.scalar.activation(p_t[:], s_t[:], Exp, accum_out=lt[:])
                    else:
                        nc.scalar.activation(p_t[:], ps_s[:], Exp, accum_out=lt[:])
                    if kt == 0:
                        nc.vector.tensor_copy(lacc[:, s : s + 1], lt[:])
                    else:
                        nc.vector.tensor_add(
                            lacc[:, s : s + 1], lacc[:, s : s + 1], lt[:]
                        )
                    # P^T via PE transpose
                    ps_pt = trps.tile([P, 512], f32r, tag="tr")
                    for kb in range(4):
                        nc.tensor.transpose(
                            ps_pt[:, kb * P : (kb + 1) * P],
                            p_t[:, kb * P : (kb + 1) * P],
                            ident[:],
                        )
                    pt_t = workp.tile([P, 512], f32r, tag="pt")
                    nc.vector.tensor_copy(pt_t[:], ps_pt[:])
                    # PV psum [q 128, f 512]
                    ps_pv = pvps.tile([P, 512], f32, tag="pv")
                    for kb in range(4):
                        nc.tensor.matmul(
                            ps_pv[:],
                            pt_t[:, kb * P : (kb + 1) * P],
                            kn_t[:, kb, :],
                            start=(kb == 0),
                            stop=(kb == 3),
                        )
                    if kt == 0:
                        nc.vector.tensor_copy(outacc[:, s, :], ps_pv[:])
                    else:
                        nc.vector.tensor_add(
                            outacc[:, s, :], outacc[:, s, :], ps_pv[:]
                        )
            for s in range(N_SLOTS):
                r_t = workp.tile([P, 1], f32, tag="lt")
                nc.vector.reciprocal(r_t[:], lacc[:, s : s + 1])
                o_t = workp.tile([P, D], f32, tag="of")
                nc.vector.tensor_scalar_mul(o_t[:], outacc[:, s, :], r_t[:])
                nc.vector.tensor_add(o_t[:], o_t[:], xq_s[:, s, :])
                nc.sync.dma_start(out_r[:, s, :], o_t[:])

    nc.compile()
    return nc


def _shard(x, W):
    """Build the 8 per-core input maps (all host-side numpy)."""
    x = np.ascontiguousarray(np.asarray(x, dtype=np.float32))
    W = np.ascontiguousarray(np.asarray(W, dtype=np.float32))
    WT = np.ascontiguousarray(W.T)
    ql = np.arange(P)[:, None]
    kl = np.arange(512)[None, :]
    in_maps = []
    for c in range(N_CORES):
        b, j = c // 4, c % 4
        blocks = [x[b, (4 * s + j) * P : (4 * s + j + 1) * P] for s in range(N_SLOTS)]
        xq = np.ascontiguousarray(np.concatenate(blocks, axis=0))  # [1024, 512]
        mask = np.where(kl <= j * P + ql, 0.0, MASK_VAL).astype(np.float32)
        in_maps.append(
            {
                "xqT": np.ascontiguousarray(xq.T),
                "xq": xq,
                "xkT": np.ascontiguousarray(x[b].T),
                "WT": WT,
                "mask": mask,
            }
        )
    return in_maps


def kernel(x, W):
    global last_exec_ns
    from concourse.bass_utils import run_bass_kernel_spmd

    if TRACE:
        _install_ntff_shim()

    if "nc" not in _CACHE:
        _CACHE["nc"] = _build()
    nc = _CACHE["nc"]

    in_maps = _shard(x, W)
    res = run_bass_kernel_spmd(
        nc, in_maps, core_ids=list(range(N_CORES)), trace=TRACE
    )
    last_exec_ns = res.exec_time_ns

    out = np.empty((B, N_CTX, D), dtype=np.float32)
    for c in range(N_CORES):
        b, j = c // 4, c % 4
        oc = res.results[c]["out"]
        for s in range(N_SLOTS):
            i = 4 * s + j
            out[b, i * P : (i + 1) * P] = oc[s * P : (s + 1) * P]
    return out


# revision 9
# speedup vs baseline: 1.2535x; 1.2535x over previous
"""Causal attention layer (K=V=x@W^T, Q=x, residual) on 8 trn2 NeuronCores.

Sharding: per batch (2), query 128-row blocks are dealt round-robin to 4
cores (core j of a batch owns blocks j, j+4, ..., j+28).  Each core runs an
identical SPMD instruction stream over 8 "slots"; slot s is the core's s-th
q-block and statically attends k-tiles 0..s (512 cols each).  The only
per-core data differences are the DMA'd inputs (its q rows + a [128,512]
additive mask for the diagonal k-tile, whose in-tile diagonal offset j*128
is slot-independent).

Algorithm per core (all matmuls f32r = full PE rate):
  K is never materialized.  Both attention products are re-associated
  through W:
    scores = x_q @ (x_k W^T)^T = (x_q W) @ x_k^T       (Y := x_q W)
    out    = P @ (x_k W^T)     = (P @ x_k) @ W^T       (Z := P @ x_k)
  Y^T is computed once in a prologue; x_k streams from DRAM in both
  layouts (x_k^T for scores rhs, x_k natural for Z rhs).  Softmax has no
  max-subtraction (scores are bounded [-75, 70]; ACT exp is accurate there
  and flushes below -88 to 0).  P^T for the Z matmul comes from PE
  transposes.  Z accumulates in SBUF over k-tiles; the epilogue applies
  Z @ W^T, the 1/l softmax normalization and the residual.
"""

import sys

import numpy as np

if "/opt/trn_rl_repo" not in sys.path:
    sys.path.insert(0, "/opt/trn_rl_repo")

B, N_CTX, D = 2, 4096, 512
P = 128
N_CORES = 8
N_SLOTS = 8  # q-blocks (128 rows) per core
N_KT = 8  # k tiles (512 cols) per batch
QROWS = N_SLOTS * P  # 1024 q rows per core
MASK_VAL = -1.0e30

_CACHE = {}

# Set to True (e.g. from test.py) to capture an NTFF profile; the measured
# max-core exec time lands in kernel.last_exec_ns.
TRACE = False
last_exec_ns = None


def _install_ntff_shim():
    """antenv.axon_hooks is absent in this image; register a stand-in so
    run_bass_kernel_spmd(trace=True) can reach the axon NTFF profiler."""
    import types

    if "antenv.axon_hooks" in sys.modules:
        return
    m = types.ModuleType("antenv.axon_hooks")
    state = {"hook": None}
    m.set_axon_ntff_profile_hook = lambda h: state.__setitem__("hook", h)
    m.get_axon_ntff_profile_hook = lambda: state["hook"]
    sys.modules["antenv.axon_hooks"] = m
    try:
        from trn_agent_boot.trn_boot import _ntff_profile_via_ctypes

        m.set_axon_ntff_profile_hook(
            _ntff_profile_via_ctypes("/opt/axon/libaxon_pjrt.so")
        )
    except Exception:
        pass


def _build():
    import concourse.mybir as mybir
    from concourse import bacc
    from concourse.tile import TileContext

    f32 = mybir.dt.float32
    f32r = mybir.dt.float32r
    Exp = mybir.ActivationFunctionType.Exp
    Copy = mybir.ActivationFunctionType.Copy
    AX = mybir.AxisListType.X

    nc = bacc.Bacc("TRN2", target_bir_lowering=False)
    xqT = nc.dram_tensor("xqT", [D, QROWS], f32r, kind="ExternalInput")
    xq = nc.dram_tensor("xq", [QROWS, D], f32, kind="ExternalInput")
    xkT = nc.dram_tensor("xkT", [D, N_CTX], f32r, kind="ExternalInput")
    xkn = nc.dram_tensor("xkn", [N_CTX, D], f32r, kind="ExternalInput")
    Wn = nc.dram_tensor("Wn", [D, D], f32r, kind="ExternalInput")  # W as [f, d]
    WT = nc.dram_tensor("WT", [D, D], f32r, kind="ExternalInput")  # W^T as [d, f]
    mask = nc.dram_tensor("mask", [P, 512], f32, kind="ExternalInput")
    out = nc.dram_tensor("out", [QROWS, D], f32, kind="ExternalOutput")

    xqT_r = xqT.rearrange("(o p) q -> p o q", p=P)  # [128, 4, 1024]
    xq_r = xq.rearrange("(s p) e -> p s e", p=P)  # [128, 8, 512]
    xkT_r = xkT.rearrange("(o p) n -> p o n", p=P)  # [128, 4, 4096]
    xkn_r = xkn.rearrange("(o p) d -> p o d", p=P)  # [128, 32, 512]
    Wn_r = Wn.rearrange("(o p) d -> p o d", p=P)  # [128, 4, 512]
    WT_r = WT.rearrange("(o p) f -> p o f", p=P)  # [128, 4, 512]
    out_r = out.rearrange("(s p) e -> p s e", p=P)

    with TileContext(nc) as tc:
        with (
            tc.tile_pool(name="const", bufs=1) as constp,
            tc.tile_pool(name="xk", bufs=3) as xkp,
            tc.tile_pool(name="work", bufs=3) as workp,
            tc.tile_pool(name="acc", bufs=1) as accp,
            tc.tile_pool(name="sc_ps", bufs=2, space="PSUM") as scps,
            tc.tile_pool(name="tr_ps", bufs=2, space="PSUM") as trps,
            tc.tile_pool(name="z_ps", bufs=2, space="PSUM") as zps,
        ):
            wn_s = constp.tile([P, 4, D], f32r)
            nc.sync.dma_start(wn_s[:], Wn_r)
            wt_s = constp.tile([P, 4, D], f32r)
            nc.sync.dma_start(wt_s[:], WT_r)
            xqT_s = constp.tile([P, 4, QROWS], f32r)
            nc.sync.dma_start(xqT_s[:], xqT_r)
            xq_s = constp.tile([P, N_SLOTS, D], f32)
            nc.sync.dma_start(xq_s[:], xq_r)
            mask_s = constp.tile([P, 512], f32)
            nc.sync.dma_start(mask_s[:], mask[:])

            from concourse.masks import make_identity

            identf = constp.tile([P, P], f32)
            make_identity(nc, identf[:])
            identr = constp.tile([P, P], f32r)
            nc.vector.tensor_copy(identr[:], identf[:])

            YT = constp.tile([P, 4, QROWS], f32r)  # (x_q W)^T resident
            zacc = accp.tile([P, N_SLOTS, D], f32)
            lacc = accp.tile([P, N_SLOTS], f32)

            # Prologue: Y^T[d, q] = sum_f W[f, d] x_q[q, f]
            for dc in range(4):
                for qh in range(2):
                    ps = scps.tile([P, 512], f32, tag="sc")
                    for fc in range(4):
                        nc.tensor.matmul(
                            ps[:],
                            wn_s[:, fc, dc * P : (dc + 1) * P],
                            xqT_s[:, fc, qh * 512 : (qh + 1) * 512],
                            start=(fc == 0),
                            stop=(fc == 3),
                        )
                    nc.vector.tensor_copy(
                        YT[:, dc, qh * 512 : (qh + 1) * 512], ps[:]
                    )

            for kt in range(N_KT):
                xkT_t = xkp.tile([P, 4, 512], f32r, tag="xkT")
                nc.sync.dma_start(xkT_t[:], xkT_r[:, :, kt * 512 : (kt + 1) * 512])
                xkn_t = xkp.tile([P, 4, 512], f32r, tag="xkn")
                nc.sync.dma_start(xkn_t[:], xkn_r[:, 4 * kt : 4 * kt + 4, :])
                for s in range(kt, N_SLOTS):
                    # scores psum [q 128, k 512] = Y[q,:] @ x_k^T
                    ps_s = scps.tile([P, 512], f32, tag="sc")
                    for dc in range(4):
                        nc.tensor.matmul(
                            ps_s[:],
                            YT[:, dc, s * P : (s + 1) * P],
                            xkT_t[:, dc, :],
                            start=(dc == 0),
                            stop=(dc == 3),
                        )
                    if s == kt:
                        nc.vector.tensor_add(ps_s[:], ps_s[:], mask_s[:])
                    # P = exp(S), straight from PSUM
                    p_t = workp.tile([P, 512], f32r, tag="p")
                    nc.scalar.activation(p_t[:], ps_s[:], Exp)
                    # l accumulation (free-axis reduce is DVE-only)
                    lt = workp.tile([P, 1], f32, tag="lt")
                    nc.vector.reduce_sum(lt[:], p_t[:], axis=AX)
                    if kt == 0:
                        nc.gpsimd.tensor_copy(lacc[:, s : s + 1], lt[:])
                    else:
                        nc.gpsimd.tensor_add(
                            lacc[:, s : s + 1], lacc[:, s : s + 1], lt[:]
                        )
                    # P^T via PE transpose, evacuated by ACT
                    ps_pt = trps.tile([P, 512], f32r, tag="tr")
                    for kb in range(4):
                        nc.tensor.transpose(
                            ps_pt[:, kb * P : (kb + 1) * P],
                            p_t[:, kb * P : (kb + 1) * P],
                            identr[:],
                        )
                    pt_t = workp.tile([P, 512], f32r, tag="pt")
                    nc.scalar.activation(pt_t[:], ps_pt[:], Copy)
                    # Z psum [q 128, d 512] = P @ x_k
                    ps_z = zps.tile([P, 512], f32, tag="z")
                    for kb in range(4):
                        nc.tensor.matmul(
                            ps_z[:],
                            pt_t[:, kb * P : (kb + 1) * P],
                            xkn_t[:, kb, :],
                            start=(kb == 0),
                            stop=(kb == 3),
                        )
                    if kt == 0:
                        nc.vector.tensor_copy(zacc[:, s, :], ps_z[:])
                    else:
                        nc.vector.tensor_add(zacc[:, s, :], zacc[:, s, :], ps_z[:])

            # Epilogue per slot: out = x_q + (Z @ W^T) / l
            for s in range(N_SLOTS):
                ps_zt = trps.tile([P, 512], f32, tag="ztr")
                for dc in range(4):
                    nc.tensor.transpose(
                        ps_zt[:, dc * P : (dc + 1) * P],
                        zacc[:, s, dc * P : (dc + 1) * P],
                        identf[:],
                    )
                zt_t = workp.tile([P, 512], f32r, tag="zt")
                nc.scalar.activation(zt_t[:], ps_zt[:], Copy)
                ps_o = zps.tile([P, 512], f32, tag="z")
                for dc in range(4):
                    nc.tensor.matmul(
                        ps_o[:],
                        zt_t[:, dc * P : (dc + 1) * P],
                        wt_s[:, dc, :],
                        start=(dc == 0),
                        stop=(dc == 3),
                    )
                r_t = workp.tile([P, 1], f32, tag="lt")
                nc.vector.reciprocal(r_t[:], lacc[:, s : s + 1])
                o_t = workp.tile([P, D], f32, tag="of")
                nc.vector.tensor_scalar_mul(o_t[:], ps_o[:], r_t[:])
                nc.vector.tensor_add(o_t[:], o_t[:], xq_s[:, s, :])
                nc.sync.dma_start(out_r[:, s, :], o_t[:])

    nc.compile()
    return nc


def _shard(x, W):
    """Build the 8 per-core input maps (all host-side numpy)."""
    x = np.ascontiguousarray(np.asarray(x, dtype=np.float32))
    W = np.ascontiguousarray(np.asarray(W, dtype=np.float32))
    WT = np.ascontiguousarray(W.T)
    ql = np.arange(P)[:, None]
    kl = np.arange(512)[None, :]
    in_maps = []
    for c in range(N_CORES):
        b, j = c // 4, c % 4
        blocks = [x[b, (4 * s + j) * P : (4 * s + j + 1) * P] for s in range(N_SLOTS)]
        xq = np.ascontiguousarray(np.concatenate(blocks, axis=0))  # [1024, 512]
        mask = np.where(kl <= j * P + ql, 0.0, MASK_VAL).astype(np.float32)
        in_maps.append(
            {
                "xqT": np.ascontiguousarray(xq.T),
                "xq": xq,
                "xkT": np.ascontiguousarray(x[b].T),
                "xkn": x[b],
                "Wn": W,
                "WT": WT,
                "mask": mask,
            }
        )
    return in_maps


def kernel(x, W):
    global last_exec_ns
    from concourse.bass_utils import run_bass_kernel_spmd

    if TRACE:
        _install_ntff_shim()

    if "nc" not in _CACHE:
        _CACHE["nc"] = _build()
    nc = _CACHE["nc"]

    in_maps = _shard(x, W)
    try:
        res = run_bass_kernel_spmd(
            nc, in_maps, core_ids=list(range(N_CORES)), trace=TRACE
        )
    except Exception:
        # one retry (transient device/profiling hiccups)
        res = run_bass_kernel_spmd(
            nc, in_maps, core_ids=list(range(N_CORES)), trace=False
        )
    last_exec_ns = res.exec_time_ns

    out = np.empty((B, N_CTX, D), dtype=np.float32)
    for c in range(N_CORES):
        b, j = c // 4, c % 4
        oc = res.results[c]["out"]
        for s in range(N_SLOTS):
            i = 4 * s + j
            out[b, i * P : (i + 1) * P] = oc[s * P : (s + 1) * P]
    return out
